# revision 9
# baseline (speedup 1.0000x reference)
"""BrightnessLoss Trainium2 kernel (raw Bass, 8-core data parallel).

reference:
    V(x)   = max_c(clip(x, 0, 1))        over channel dim (RGB)
    result = mean(|V(pred) - V(target)|) over (N, H, W)

Identities used on device:
    clip(max(r,g,b),0,1) == max_c(clip(x,0,1))          (clip is monotone)
    W := relu(1 - relu(m)) == 1 - clip(m, 0, 1)
    |Vp - Vt| == |Wp - Wt|
    sum|Wp - Wt| == 2*sum max(Wp,Wt) - sum(Wp + Wt)

The stream is the roofline: ~25.2 MB of fp32 input per core, and the 16
SDMA engines cap at ~24 GB/s each with 4 KB packets (per-packet
overhead), a bit more with 8 KB packets. So the design goal is a gapless
two-ring DMA stream of the largest-possible contiguous runs, with compute
strictly faster than arrival:

  - DMA "groups" cover column ranges of each image; pred rides the Sync
    HWDGE ring, targ rides the ACT HWDGE ring (12.6 MB each, symmetric).
    Images 1 and 2 go as single full-row groups (8 KB DRAM runs); image 0
    leads with small groups (128/256/640 cols) so compute starts ~1 us
    into the stream; image 3 trails with small groups (640/256/128) so
    the closing dependency chain is short.
  - 4 group slots [P, 2, 3, w] (both sides side-by-side) are queued
    upfront on both rings (~8.3 MB deep), so the rings never run dry.
  - Compute "units" (<=1024 cols) subdivide groups. Per unit, both sides
    in one wide op:
        DVE TT   m = max(R2, G2)          [P, 2, w]
        DVE STT  u = max(max(m,0), B2)    [P, 2, w]
        ACT      W = Relu(1 - u) (bf16),  accum_out = sum(Wp)+sum(Wt)
        DVE STT  max(Wp, Wt) (bf16),      accum_out = sum
    DVE needs ~5.8 us per 1024-col unit vs ~7.3 us arrival, so it stays
    caught up and the tail after the last packet is just the last small
    unit's chain. Partials go out in two DMAs (bulk early, last units at
    the end). Host combines in float64.
"""

import numpy as np

N_CORES = 8
N_IMG = 4  # 32 / 8
C = 3
P = 128
F = 2048  # 512*512 / 128
N_PIX = 32 * 512 * 512
FC = 1024  # max compute-unit width
S_G = 4  # group slot depth
HEAD_SPLIT = (128, 256, 640, 1024)  # image 0 groups (sum = F)
TAIL_SPLIT = (1024, 640, 256, 128)  # last image groups (sum = F)


def _plan():
    """groups: (img, col_off, width); units: (grp_idx, off_in_grp, width)."""
    groups = []
    o = 0
    for w in HEAD_SPLIT:
        groups.append((0, o, w))
        o += w
    assert o == F
    for img in range(1, N_IMG - 1):
        groups.append((img, 0, F))
    o = 0
    for w in TAIL_SPLIT:
        groups.append((N_IMG - 1, o, w))
        o += w
    assert o == F
    units = []
    for g, (_img, _off, w) in enumerate(groups):
        o = 0
        while o < w:
            uw = min(FC, w - o)
            units.append((g, o, uw))
            o += uw
    return groups, units


def _build_program():
    from contextlib import ExitStack

    import concourse.bass as bass
    import concourse.mybir as mybir

    fp32 = mybir.dt.float32
    bf16 = mybir.dt.bfloat16
    Alu = mybir.AluOpType
    Act = mybir.ActivationFunctionType

    groups, units = _plan()
    n_groups = len(groups)
    n_units = len(units)
    last_unit_of = {}
    for u, (g, _o, _w) in enumerate(units):
        last_unit_of[g] = u
    slot_w = [
        max(groups[g][2] for g in range(s, n_groups, S_G)) for s in range(S_G)
    ]

    # detect_race_conditions=False: the raw-mode CoreSim race detector can't
    # see same-engine program-order (DVE m1 -> STT RAW); hardware engines
    # execute in order.
    # The construction-time all_engine_barrier orders the const-tile memsets
    # against engines that read them; this kernel uses only instruction
    # immediates, so skip it and let the engines reach first work sooner.
    _orig_barrier = bass.Bass.all_engine_barrier
    bass.Bass.all_engine_barrier = lambda *a, **k: None
    try:
        nc = bass.Bass(
            "TRN2",
            target_bir_lowering=False,
            debug=False,
            detect_race_conditions=False,
        )
    finally:
        bass.Bass.all_engine_barrier = _orig_barrier
    pred = nc.dram_tensor("pred", [N_IMG, C, P, F], fp32, kind="ExternalInput").ap()
    targ = nc.dram_tensor("target", [N_IMG, C, P, F], fp32, kind="ExternalInput").ap()
    out = nc.dram_tensor(
        "partials", [P, 2 * n_units], fp32, kind="ExternalOutput"
    ).ap()

    with ExitStack() as ctx:
        sb = lambda name, shape, dt=fp32: ctx.enter_context(
            nc.sbuf_tensor(name, shape, dt)
        )
        sem = lambda name: ctx.enter_context(nc.semaphore(name))

        # one slot holds BOTH sides of a group: [P, side, chan, slot_w]
        inb = [sb(f"in{s}", [P, 2, C, slot_w[s]]) for s in range(S_G)]
        ub = [sb(f"u{s}", [P, 2 * FC]) for s in range(2)]
        wb = [sb(f"w{s}", [P, 2 * FC], bf16) for s in range(2)]
        m1 = sb("m1", [P, 2 * FC])
        scr = sb("stt_scratch", [P, FC], bf16)
        acc = sb("acc", [P, 2 * n_units])

        inp_sem = sem("inp")  # +16 per pred group (Sync ring, in order)
        int_sem = sem("int")  # +16 per targ group (ACT ring, in order)
        u_sem = sem("u")  # +1 per unit after DVE STT (inb consumed)
        act_sem = sem("act")  # +1 per unit after ACT (ub consumed, wb+acc ready)
        gp_sem = sem("gp")  # +1 per unit after DVE accum (wb consumed)
        out_sem = sem("outd")

        # WAR gate for issuing group g into slot g%S_G: the last unit of
        # group g-S_G must have been consumed by DVE's STT.
        def war_level(g):
            return last_unit_of[g - S_G] + 1 if g >= S_G else 0

        def dma_in(eng, side_idx, g):
            img, off, w = groups[g]
            side = (pred, targ)[side_idx]
            s_sem = (inp_sem, int_sem)[side_idx]
            src = side[img, :, :, off : off + w].rearrange("c p f -> p c f")
            eng.dma_start(
                out=inb[g % S_G][:, side_idx, :, :w],
                in_=src,
            ).then_inc(s_sem, 16)

        block = ctx.enter_context(nc.Block(no_gpsimd_drain=True))

        @block.sync
        def _(sync):
            # the whole pred side rides the SP HWDGE ring, queued as deep as
            # the slots allow (4 groups ~ 8.3 MB upfront)
            for g in range(n_groups):
                lvl = war_level(g)
                if lvl:
                    sync.wait_ge(u_sem, lvl)
                dma_in(sync, 0, g)
            if n_units > 2:
                # bulk of partials early; only the last 2 units' cols remain.
                # gp_sem >= k implies act_sem >= k (accum u waits ACT u), so
                # both engines' acc columns for units < k are final.
                sync.wait_ge(gp_sem, n_units - 2)
                sync.dma_start(
                    out=out[:, : 2 * (n_units - 2)],
                    in_=acc[:, : 2 * (n_units - 2)],
                ).then_inc(out_sem, 16)
            sync.wait_ge(gp_sem, n_units)
            # No out_sem wait after the final write: the block-exit drain
            # fences the HWDGE ring before NEFF completion.
            sync.dma_start(
                out=out[:, 2 * max(0, n_units - 2) :],
                in_=acc[:, 2 * max(0, n_units - 2) :],
            ).then_inc(out_sem, 16)

        @block.vector
        def _(vector):
            def accum(u):
                # max(Wp, Wt) elementwise (bf16), accum_out = row sum
                w = units[u][2]
                vector.wait_ge(act_sem, u + 1)
                vector.scalar_tensor_tensor(
                    scr[:, :w],
                    wb[u % 2][:, :w],
                    0.0,
                    wb[u % 2][:, w : 2 * w],
                    op0=Alu.bypass,
                    op1=Alu.max,
                    accum_out=acc[:, 2 * u : 2 * u + 1],
                ).then_inc(gp_sem, 1)

            for u in range(n_units):
                g, o, w = units[u]
                t = inb[g % S_G]
                vector.wait_ge(inp_sem, 16 * (g + 1))
                vector.wait_ge(int_sem, 16 * (g + 1))
                mv = m1[:, : 2 * w].rearrange("p (s w) -> p s w", s=2)
                uv = ub[u % 2][:, : 2 * w].rearrange("p (s w) -> p s w", s=2)
                vector.tensor_max(
                    mv, t[:, :, 0, o : o + w], t[:, :, 1, o : o + w]
                )
                if u >= 2:
                    # WAR on ub[u%2]: ACT's W of unit u-2 (its reader)
                    vector.wait_ge(act_sem, u - 1)
                vector.scalar_tensor_tensor(
                    uv,
                    mv,
                    0.0,
                    t[:, :, 2, o : o + w],
                    op0=Alu.max,
                    op1=Alu.max,
                ).then_inc(u_sem, 1)
                if u > 0:
                    accum(u - 1)
            accum(n_units - 1)

        @block.scalar
        def _(scalar):
            # the whole targ side rides the ACT HWDGE ring. Groups whose WAR
            # level is already implied by the previous ACT's u_sem wait are
            # issued between activations; the first S_G go up front.
            issue_after = {}  # ACT index n -> groups to issue after it
            for g in range(S_G, n_groups):
                # after ACT(n) with n+1 >= war_level(g): earliest n
                n = war_level(g) - 1
                issue_after.setdefault(n, []).append(g)
            for g in range(min(S_G, n_groups)):
                dma_in(scalar, 1, g)
            for n in range(n_units):
                w = units[n][2]
                scalar.wait_ge(u_sem, n + 1)
                if n >= 2:
                    # WAR on wb[n%2]: accum of unit n-2 (its reader)
                    scalar.wait_ge(gp_sem, n - 1)
                scalar.activation(
                    wb[n % 2][:, : 2 * w],
                    ub[n % 2][:, : 2 * w],
                    Act.Relu,
                    bias=1.0,
                    scale=-1.0,
                    accum_out=acc[:, 2 * n + 1 : 2 * n + 2],
                ).then_inc(act_sem, 1)
                for g in issue_after.get(n, ()):
                    dma_in(scalar, 1, g)

        # Skip the Block-exit all-engine barrier (~4.3us): every cross-engine
        # dependency is semaphore-gated and the per-engine exit drains
        # (no_gpsimd_drain path) still fence the DMA rings, so engines may
        # halt independently — NEFF completion waits for all engines anyway.
        nc.all_engine_barrier = lambda *a, **k: None

    del nc.all_engine_barrier  # restore class method
    return nc


_program = None


def _get_program():
    global _program
    if _program is None:
        _program = _build_program()
    return _program


def _finish(partials_list):
    """partials_list: per-core [P, 2*n_units] f32 with cols per unit:
    [sum max(Wp,Wt), sum Wp + sum Wt].
    sum|Vp-Vt| = 2*sum(max) - (sum Wp + sum Wt)."""
    total = np.float64(0.0)
    for p in partials_list:
        p = p.astype(np.float64)
        total += 2.0 * p[:, 0::2].sum() - p[:, 1::2].sum()
    return np.array(total / N_PIX, dtype=np.float32)


def kernel(pred: np.ndarray, target: np.ndarray) -> np.ndarray:
    from concourse.bass_utils import run_bass_kernel_spmd

    nc = _get_program()
    pred = np.ascontiguousarray(pred, dtype=np.float32).reshape(
        N_CORES, N_IMG, C, P, F
    )
    target = np.ascontiguousarray(target, dtype=np.float32).reshape(
        N_CORES, N_IMG, C, P, F
    )
    in_maps = [{"pred": pred[i], "target": target[i]} for i in range(N_CORES)]
    res = run_bass_kernel_spmd(nc, in_maps, list(range(N_CORES)))
    return _finish([r["partials"] for r in res.results])
